# revision 1
# baseline (speedup 1.0000x reference)
"""Trainium2 Bass kernel for the LGP-instruction module (read -> op bank -> write).

Data-parallel over batch: core b computes x[b] (2048, 4096).
Device pipeline per core:
  phase 1: valuesT[C, T] = sum_vt rw_tile[vt].T @ xT_tile[vt]   (PSUM, 4 banks)
  phase 2: per T-chunk of 512:
     h_i = W_i.T @ valuesT  (PSUM) -> ACT f_i(h + b_i) -> DVE weighted-accumulate
     out[Tsub, V] = accT.T @ wwT  -> DVE copy -> DMA store
Host prep: read_w softmax, write_w*out_scale transpose, x[b].T layout.
Matmuls run as float32r (fp32 bits, full-rate PE streaming).
"""
import sys
import numpy as np

if '/opt/trn_rl_repo' not in sys.path:
    sys.path.insert(0, '/opt/trn_rl_repo')

B, T, V, C, NOPS = 8, 2048, 4096, 128, 8
NCORES = 8
NV = V // 128     # 32 v-tiles
NTC = T // 512    # 4 T-chunks

_CACHE = {}
LAST_RESULT = None


def _build(pre, post):
    from concourse import bass, bacc, tile, mybir
    f32, f32r = mybir.dt.float32, mybir.dt.float32r
    AF = mybir.ActivationFunctionType
    ts = bass.ts
    FUNCS = [AF.Identity, AF.Relu, AF.Gelu, AF.Square,
             AF.Identity, AF.Abs, AF.Tanh, AF.Sigmoid]

    nc = bacc.Bacc("TRN2", target_bir_lowering=False, debug=False,
                   num_devices=NCORES)
    xT = nc.dram_tensor("xT", [V, T], f32r, kind="ExternalInput")
    rw = nc.dram_tensor("rw", [V, C], f32r, kind="ExternalInput")
    wwT = nc.dram_tensor("wwT", [C, V], f32r, kind="ExternalInput")
    opw = nc.dram_tensor("opw", [NOPS, C, C], f32r, kind="ExternalInput")
    opb = nc.dram_tensor("opb", [C, NOPS], f32, kind="ExternalInput")
    out = nc.dram_tensor("out", [T, V], f32, kind="ExternalOutput")

    with tile.TileContext(nc) as tc:
        with tc.tile_pool(name="const", bufs=1) as constp, \
             tc.tile_pool(name="xt", bufs=3) as xtp, \
             tc.tile_pool(name="vals_ps", bufs=1, space="PSUM") as vpsp, \
             tc.tile_pool(name="vals_sb", bufs=1) as vsbp, \
             tc.tile_pool(name="h_ps", bufs=2, space="PSUM") as hpsp, \
             tc.tile_pool(name="t_sb", bufs=3) as tp, \
             tc.tile_pool(name="acc", bufs=2) as accp, \
             tc.tile_pool(name="out_ps", bufs=2, space="PSUM") as opsp, \
             tc.tile_pool(name="out_sb", bufs=2) as osbp:

            rw_t = constp.tile([128, NV, C], f32r)
            nc.sync.dma_start(rw_t[:], rw.ap().rearrange("(vt p) c -> p vt c", p=128))
            wwT_t = constp.tile([C, V], f32r)
            nc.sync.dma_start(wwT_t[:], wwT.ap())
            opw_t = constp.tile([C, NOPS, C], f32r)
            nc.sync.dma_start(opw_t[:], opw.ap().rearrange("i p c -> p i c"))
            opb_t = constp.tile([C, NOPS], f32)
            nc.sync.dma_start(opb_t[:], opb.ap())

            # phase 1: contraction over V into 4 psum banks
            values = vpsp.tile([128, T], f32)
            for vt in range(NV):
                xt = xtp.tile([128, T], f32r)
                nc.sync.dma_start(xt[:], xT.ap()[ts(vt, 128), :])
                for tcn in range(NTC):
                    nc.tensor.matmul(values[:, ts(tcn, 512)], rw_t[:, vt, :],
                                     xt[:, ts(tcn, 512)],
                                     start=(vt == 0), stop=(vt == NV - 1))
            vals = vsbp.tile([128, T], f32r)
            for tcn in range(NTC):
                nc.vector.tensor_copy(vals[:, ts(tcn, 512)], values[:, ts(tcn, 512)])

            # phase 2
            for tcn in range(NTC):
                acc = accp.tile([128, 512], f32r)
                for i in range(NOPS):
                    h = hpsp.tile([128, 512], f32)
                    nc.tensor.matmul(h[:], opw_t[:, i, :], vals[:, ts(tcn, 512)],
                                     start=True, stop=True)
                    if i == 0:
                        nc.scalar.activation(acc[:], h[:], FUNCS[0],
                                             bias=opb_t[:, 0:1], scale=pre[0])
                    else:
                        t = tp.tile([128, 512], f32r)
                        nc.scalar.activation(t[:], h[:], FUNCS[i],
                                             bias=opb_t[:, i:i + 1], scale=pre[i])
                        nc.vector.scalar_tensor_tensor(
                            acc[:], t[:], post[i], acc[:],
                            op0=mybir.AluOpType.mult, op1=mybir.AluOpType.add)
                for sub in range(4):
                    osb = osbp.tile([128, V], f32)
                    for nn in range(8):
                        ops_ = opsp.tile([128, 512], f32)
                        nc.tensor.matmul(ops_[:], acc[:, ts(sub, 128)],
                                         wwT_t[:, ts(nn, 512)],
                                         start=True, stop=True)
                        nc.vector.tensor_copy(osb[:, ts(nn, 512)], ops_[:])
                    nc.sync.dma_start(out.ap()[ts(tcn * 4 + sub, 128), :], osb[:])
    nc.compile()
    return nc


def _softmax(x, axis):
    x = np.asarray(x, np.float32)
    m = x.max(axis=axis, keepdims=True)
    e = np.exp(x - m)
    return e / e.sum(axis=axis, keepdims=True)


def kernel(x, basis, read_coeffs, write_coeffs, op_logits, op_weights,
           op_biases, out_scale):
    global LAST_RESULT
    from concourse.bass_utils import run_bass_kernel_spmd

    x = np.asarray(x, np.float32)
    basis = np.asarray(basis, np.float32)
    read_coeffs = np.asarray(read_coeffs, np.float32)
    write_coeffs = np.asarray(write_coeffs, np.float32)
    op_logits = np.asarray(op_logits, np.float32)
    op_weights = np.asarray(op_weights, np.float32)
    op_biases = np.asarray(op_biases, np.float32)
    out_scale = np.float32(out_scale)

    read_w = _softmax(basis @ read_coeffs.T, axis=0)               # (V, C)
    wwT = np.ascontiguousarray((basis @ write_coeffs.T).T) * out_scale  # (C, V)
    w = _softmax(op_logits, axis=0).astype(np.float64)

    # fold the mixture weight into ACT scale/bias where the nonlinearity allows
    #   i: 0 ident, 1 relu, 2 gelu, 3 square, 4 neg, 5 abs, 6 tanh, 7 sigmoid
    pre = [w[0], w[1], 1.0, np.sqrt(w[3]), -w[4], w[5], 1.0, 1.0]
    post = [1.0, 1.0, w[2], 1.0, 1.0, 1.0, w[6], w[7]]
    pre = [float(v) for v in pre]
    post = [float(v) for v in post]

    key = tuple(pre) + tuple(post)
    if key not in _CACHE:
        _CACHE[key] = _build(pre, post)
    nc = _CACHE[key]

    opb = (op_biases.T * np.array(pre, np.float64)[None, :]).astype(np.float32)
    # gelu/tanh/sigmoid biases enter before the nonlinearity unscaled
    for i in (2, 6, 7):
        opb[:, i] = op_biases[i]

    shared = {
        "rw": read_w,
        "wwT": wwT.astype(np.float32),
        "opw": op_weights,
        "opb": np.ascontiguousarray(opb),
    }
    in_maps = []
    for b in range(B):
        m = dict(shared)
        m["xT"] = np.ascontiguousarray(x[b].T)
        in_maps.append(m)

    res = run_bass_kernel_spmd(nc, in_maps, core_ids=list(range(NCORES)))
    LAST_RESULT = res
    out = np.empty((B, T, V), np.float32)
    for b in range(B):
        out[b] = res.results[b]["out"]
    return out


# revision 3
# speedup vs baseline: 1.0744x; 1.0744x over previous
"""Trainium2 Bass kernel for the LGP-instruction module (read -> op bank -> write).

Data-parallel over batch: core b computes x[b] (2048, 4096).
Device pipeline per core:
  phase 1: valuesT[C, T] = sum_vt rw_tile[vt].T @ xT_tile[vt]   (PSUM, 4 banks)
  phase 2: per T-chunk of 512:
     h_i = W_i.T @ valuesT  (PSUM) -> ACT f_i(h + b_i) -> DVE weighted-accumulate
     out[Tsub, V] = accT.T @ wwT  -> DVE copy -> DMA store
Host prep: read_w softmax, write_w*out_scale transpose, x[b].T layout.
Matmuls run as float32r (fp32 bits, full-rate PE streaming).
"""
import sys
import numpy as np

if '/opt/trn_rl_repo' not in sys.path:
    sys.path.insert(0, '/opt/trn_rl_repo')

B, T, V, C, NOPS = 8, 2048, 4096, 128, 8
NCORES = 8
NV = V // 128     # 32 v-tiles
NTC = T // 512    # 4 T-chunks

_CACHE = {}
LAST_RESULT = None


def _build(pre, post):
    from concourse import bass, bacc, tile, mybir
    f32, f32r = mybir.dt.float32, mybir.dt.float32r
    AF = mybir.ActivationFunctionType
    ts = bass.ts
    FUNCS = [AF.Identity, AF.Relu, AF.Gelu, AF.Square,
             AF.Identity, AF.Abs, AF.Tanh, AF.Sigmoid]

    nc = bacc.Bacc("TRN2", target_bir_lowering=False, debug=False,
                   num_devices=NCORES)
    xT = nc.dram_tensor("xT", [V, T], f32r, kind="ExternalInput")
    rw = nc.dram_tensor("rw", [V, C], f32r, kind="ExternalInput")
    wwT = nc.dram_tensor("wwT", [C, V], f32r, kind="ExternalInput")
    opw = nc.dram_tensor("opw", [NOPS, C, C], f32r, kind="ExternalInput")
    opb = nc.dram_tensor("opb", [C, NOPS], f32, kind="ExternalInput")
    out = nc.dram_tensor("out", [T, V], f32, kind="ExternalOutput")

    NBLK = 4          # xT load blocks per T-chunk
    VB = NV // NBLK   # 8 v-tiles per block

    # xT viewed as [p, vtile, t]
    xTr = xT.ap().rearrange("(vb p) t -> p vb t", p=128)

    with tile.TileContext(nc) as tc:
        with tc.tile_pool(name="const", bufs=1) as constp, \
             tc.tile_pool(name="xt", bufs=6) as xtp, \
             tc.tile_pool(name="vals_ps", bufs=2, space="PSUM") as vpsp, \
             tc.tile_pool(name="vals_sb", bufs=2) as vsbp, \
             tc.tile_pool(name="h_ps", bufs=3, space="PSUM") as hpsp, \
             tc.tile_pool(name="t_sb", bufs=3) as tp, \
             tc.tile_pool(name="acc", bufs=2) as accp, \
             tc.tile_pool(name="out_ps", bufs=3, space="PSUM") as opsp, \
             tc.tile_pool(name="out_sb", bufs=2) as osbp:

            rw_t = constp.tile([128, NV, C], f32r)
            nc.sync.dma_start(rw_t[:], rw.ap().rearrange("(vt p) c -> p vt c", p=128))
            wwT_t = constp.tile([C, V], f32r)
            nc.sync.dma_start(wwT_t[:], wwT.ap())
            opw_t = constp.tile([C, NOPS, C], f32r)
            nc.sync.dma_start(opw_t[:], opw.ap().rearrange("i p c -> p i c"))
            opb_t = constp.tile([C, NOPS], f32)
            nc.sync.dma_start(opb_t[:], opb.ap())

            for tcn in range(NTC):
                # read: accumulate over all V into one psum bank
                values = vpsp.tile([128, 512], f32)
                for blk in range(NBLK):
                    xt = xtp.tile([128, VB, 512], f32r)
                    nc.sync.dma_start(
                        xt[:], xTr[:, ts(blk, VB), ts(tcn, 512)])
                    for j in range(VB):
                        vt = blk * VB + j
                        nc.tensor.matmul(values[:], rw_t[:, vt, :], xt[:, j, :],
                                         start=(vt == 0), stop=(vt == NV - 1))
                vals = vsbp.tile([128, 512], f32r)
                nc.vector.tensor_copy(vals[:], values[:])

                # op bank
                acc = accp.tile([128, 512], f32r)
                for i in range(NOPS):
                    h = hpsp.tile([128, 512], f32)
                    nc.tensor.matmul(h[:], opw_t[:, i, :], vals[:],
                                     start=True, stop=True)
                    if i == 0:
                        nc.scalar.activation(acc[:], h[:], FUNCS[0],
                                             bias=opb_t[:, 0:1], scale=pre[0])
                    else:
                        t = tp.tile([128, 512], f32r)
                        nc.scalar.activation(t[:], h[:], FUNCS[i],
                                             bias=opb_t[:, i:i + 1], scale=pre[i])
                        nc.vector.scalar_tensor_tensor(
                            acc[:], t[:], post[i], acc[:],
                            op0=mybir.AluOpType.mult, op1=mybir.AluOpType.add)

                # write: out rows, stores on SWDGE so loads never queue behind them
                for sub in range(4):
                    osb = osbp.tile([128, V], f32)
                    for nn in range(8):
                        ops_ = opsp.tile([128, 512], f32)
                        nc.tensor.matmul(ops_[:], acc[:, ts(sub, 128)],
                                         wwT_t[:, ts(nn, 512)],
                                         start=True, stop=True)
                        nc.vector.tensor_copy(osb[:, ts(nn, 512)], ops_[:])
                    nc.gpsimd.dma_start(out.ap()[ts(tcn * 4 + sub, 128), :], osb[:])
    nc.compile()
    return nc


def _softmax(x, axis):
    x = np.asarray(x, np.float32)
    m = x.max(axis=axis, keepdims=True)
    e = np.exp(x - m)
    return e / e.sum(axis=axis, keepdims=True)


def kernel(x, basis, read_coeffs, write_coeffs, op_logits, op_weights,
           op_biases, out_scale):
    global LAST_RESULT
    from concourse.bass_utils import run_bass_kernel_spmd

    x = np.asarray(x, np.float32)
    basis = np.asarray(basis, np.float32)
    read_coeffs = np.asarray(read_coeffs, np.float32)
    write_coeffs = np.asarray(write_coeffs, np.float32)
    op_logits = np.asarray(op_logits, np.float32)
    op_weights = np.asarray(op_weights, np.float32)
    op_biases = np.asarray(op_biases, np.float32)
    out_scale = np.float32(out_scale)

    read_w = _softmax(basis @ read_coeffs.T, axis=0)               # (V, C)
    wwT = np.ascontiguousarray((basis @ write_coeffs.T).T) * out_scale  # (C, V)
    w = _softmax(op_logits, axis=0).astype(np.float64)

    # fold the mixture weight into ACT scale/bias where the nonlinearity allows
    #   i: 0 ident, 1 relu, 2 gelu, 3 square, 4 neg, 5 abs, 6 tanh, 7 sigmoid
    pre = [w[0], w[1], 1.0, np.sqrt(w[3]), -w[4], w[5], 1.0, 1.0]
    post = [1.0, 1.0, w[2], 1.0, 1.0, 1.0, w[6], w[7]]
    pre = [float(v) for v in pre]
    post = [float(v) for v in post]

    key = tuple(pre) + tuple(post)
    if key not in _CACHE:
        _CACHE[key] = _build(pre, post)
    nc = _CACHE[key]

    opb = (op_biases.T * np.array(pre, np.float64)[None, :]).astype(np.float32)
    # gelu/tanh/sigmoid biases enter before the nonlinearity unscaled
    for i in (2, 6, 7):
        opb[:, i] = op_biases[i]

    shared = {
        "rw": read_w,
        "wwT": wwT.astype(np.float32),
        "opw": op_weights,
        "opb": np.ascontiguousarray(opb),
    }
    in_maps = []
    for b in range(B):
        m = dict(shared)
        m["xT"] = np.ascontiguousarray(x[b].T)
        in_maps.append(m)

    res = run_bass_kernel_spmd(nc, in_maps, core_ids=list(range(NCORES)))
    LAST_RESULT = res
    out = np.empty((B, T, V), np.float32)
    for b in range(B):
        out[b] = res.results[b]["out"]
    return out


# revision 5
# speedup vs baseline: 1.1199x; 1.0423x over previous
"""Trainium2 Bass kernel for the LGP-instruction module (read -> op bank -> write).

Data-parallel over batch: core b computes x[b] (2048, 4096).
Device pipeline per core:
  phase 1: valuesT[C, T] = sum_vt rw_tile[vt].T @ xT_tile[vt]   (PSUM, 4 banks)
  phase 2: per T-chunk of 512:
     h_i = W_i.T @ valuesT  (PSUM) -> ACT f_i(h + b_i) -> DVE weighted-accumulate
     out[Tsub, V] = accT.T @ wwT  -> DVE copy -> DMA store
Host prep: read_w softmax, write_w*out_scale transpose, x[b].T layout.
Matmuls run as float32r (fp32 bits, full-rate PE streaming).
"""
import sys
import numpy as np

if '/opt/trn_rl_repo' not in sys.path:
    sys.path.insert(0, '/opt/trn_rl_repo')

B, T, V, C, NOPS = 8, 2048, 4096, 128, 8
NCORES = 8
NV = V // 128     # 32 v-tiles
NTC = T // 512    # 4 T-chunks

_CACHE = {}
LAST_RESULT = None


def _build(pre, post):
    from concourse import bass, bacc, tile, mybir
    f32, f32r = mybir.dt.float32, mybir.dt.float32r
    AF = mybir.ActivationFunctionType
    ts = bass.ts
    FUNCS = [AF.Identity, AF.Relu, AF.Gelu, AF.Square,
             AF.Identity, AF.Abs, AF.Tanh, AF.Sigmoid]

    nc = bacc.Bacc("TRN2", target_bir_lowering=False, debug=False,
                   num_devices=NCORES)
    xT = nc.dram_tensor("xT", [V, T], f32r, kind="ExternalInput")
    rw = nc.dram_tensor("rw", [V, C], f32r, kind="ExternalInput")
    wwT = nc.dram_tensor("wwT", [C, V], f32r, kind="ExternalInput")
    opw = nc.dram_tensor("opw", [NOPS, C, C], f32r, kind="ExternalInput")
    opb = nc.dram_tensor("opb", [C, NOPS], f32, kind="ExternalInput")
    out = nc.dram_tensor("out", [T, V], f32, kind="ExternalOutput")

    NBLK = 4          # xT load blocks per T-chunk
    VB = NV // NBLK   # 8 v-tiles per block

    # xT viewed as [p, vtile, t]
    xTr = xT.ap().rearrange("(vb p) t -> p vb t", p=128)

    with tile.TileContext(nc) as tc:
        with tc.tile_pool(name="const", bufs=1) as constp, \
             tc.tile_pool(name="xt", bufs=6) as xtp, \
             tc.tile_pool(name="vals_ps", bufs=2, space="PSUM") as vpsp, \
             tc.tile_pool(name="vals_sb", bufs=2) as vsbp, \
             tc.tile_pool(name="h_ps", bufs=3, space="PSUM") as hpsp, \
             tc.tile_pool(name="t_sb", bufs=3) as tp, \
             tc.tile_pool(name="acc", bufs=2) as accp, \
             tc.tile_pool(name="out_ps", bufs=3, space="PSUM") as opsp, \
             tc.tile_pool(name="out_sb", bufs=2) as osbp:

            rw_t = constp.tile([128, NV, C], f32r)
            nc.sync.dma_start(rw_t[:], rw.ap().rearrange("(vt p) c -> p vt c", p=128))
            wwT_t = constp.tile([C, V], f32r)
            nc.sync.dma_start(wwT_t[:], wwT.ap())
            opw_t = constp.tile([C, NOPS, C], f32r)
            nc.sync.dma_start(opw_t[:], opw.ap().rearrange("i p c -> p i c"))
            opb_t = constp.tile([C, NOPS], f32)
            nc.sync.dma_start(opb_t[:], opb.ap())

            for tcn in range(NTC):
                # read: accumulate over all V into one psum bank
                values = vpsp.tile([128, 512], f32)
                for blk in range(NBLK):
                    xt = xtp.tile([128, VB, 512], f32r)
                    nc.sync.dma_start(
                        xt[:], xTr[:, ts(blk, VB), ts(tcn, 512)])
                    for j in range(VB):
                        vt = blk * VB + j
                        nc.tensor.matmul(values[:], rw_t[:, vt, :], xt[:, j, :],
                                         start=(vt == 0), stop=(vt == NV - 1))
                vals = vsbp.tile([128, 512], f32r)
                nc.vector.tensor_copy(vals[:], values[:])

                # op bank
                acc = accp.tile([128, 512], f32r)
                for i in range(NOPS):
                    h = hpsp.tile([128, 512], f32)
                    nc.tensor.matmul(h[:], opw_t[:, i, :], vals[:],
                                     start=True, stop=True)
                    if i == 0:
                        nc.scalar.activation(acc[:], h[:], FUNCS[0],
                                             bias=opb_t[:, 0:1], scale=pre[0])
                    else:
                        t = tp.tile([128, 512], f32r)
                        nc.scalar.activation(t[:], h[:], FUNCS[i],
                                             bias=opb_t[:, i:i + 1], scale=pre[i])
                        nc.vector.scalar_tensor_tensor(
                            acc[:], t[:], post[i], acc[:],
                            op0=mybir.AluOpType.mult, op1=mybir.AluOpType.add)

                # write: out rows, stores on SWDGE so loads never queue behind them
                for sub in range(4):
                    osb = osbp.tile([128, V], f32)
                    for nn in range(8):
                        ops_ = opsp.tile([128, 512], f32)
                        nc.tensor.matmul(ops_[:], acc[:, ts(sub, 128)],
                                         wwT_t[:, ts(nn, 512)],
                                         start=True, stop=True)
                        idx = (tcn * 4 + sub) * 8 + nn
                        if idx % 9 < 2:   # ~2/9 of psum-drain copies go to ACT
                            nc.scalar.copy(osb[:, ts(nn, 512)], ops_[:])
                        else:
                            nc.vector.tensor_copy(osb[:, ts(nn, 512)], ops_[:])
                    nc.gpsimd.dma_start(out.ap()[ts(tcn * 4 + sub, 128), :], osb[:])
    nc.compile()
    return nc


def _softmax(x, axis):
    x = np.asarray(x, np.float32)
    m = x.max(axis=axis, keepdims=True)
    e = np.exp(x - m)
    return e / e.sum(axis=axis, keepdims=True)


def kernel(x, basis, read_coeffs, write_coeffs, op_logits, op_weights,
           op_biases, out_scale):
    global LAST_RESULT
    from concourse.bass_utils import run_bass_kernel_spmd

    x = np.asarray(x, np.float32)
    basis = np.asarray(basis, np.float32)
    read_coeffs = np.asarray(read_coeffs, np.float32)
    write_coeffs = np.asarray(write_coeffs, np.float32)
    op_logits = np.asarray(op_logits, np.float32)
    op_weights = np.asarray(op_weights, np.float32)
    op_biases = np.asarray(op_biases, np.float32)
    out_scale = np.float32(out_scale)

    read_w = _softmax(basis @ read_coeffs.T, axis=0)               # (V, C)
    wwT = np.ascontiguousarray((basis @ write_coeffs.T).T) * out_scale  # (C, V)
    w = _softmax(op_logits, axis=0).astype(np.float64)

    # fold the mixture weight into ACT scale/bias where the nonlinearity allows
    #   i: 0 ident, 1 relu, 2 gelu, 3 square, 4 neg, 5 abs, 6 tanh, 7 sigmoid
    pre = [w[0], w[1], 1.0, np.sqrt(w[3]), -w[4], w[5], 1.0, 1.0]
    post = [1.0, 1.0, w[2], 1.0, 1.0, 1.0, w[6], w[7]]
    pre = [float(v) for v in pre]
    post = [float(v) for v in post]

    key = tuple(pre) + tuple(post)
    if key not in _CACHE:
        _CACHE[key] = _build(pre, post)
    nc = _CACHE[key]

    opb = (op_biases.T * np.array(pre, np.float64)[None, :]).astype(np.float32)
    # gelu/tanh/sigmoid biases enter before the nonlinearity unscaled
    for i in (2, 6, 7):
        opb[:, i] = op_biases[i]

    shared = {
        "rw": read_w,
        "wwT": wwT.astype(np.float32),
        "opw": op_weights,
        "opb": np.ascontiguousarray(opb),
    }
    in_maps = []
    for b in range(B):
        m = dict(shared)
        m["xT"] = np.ascontiguousarray(x[b].T)
        in_maps.append(m)

    res = run_bass_kernel_spmd(nc, in_maps, core_ids=list(range(NCORES)))
    LAST_RESULT = res
    out = np.empty((B, T, V), np.float32)
    for b in range(B):
        out[b] = res.results[b]["out"]
    return out
